# revision 33
# baseline (speedup 1.0000x reference)
"""Trainium2 Bass kernel for a 2-layer leaky-integrate-and-fire SNN.

Model (per timestep t, snnTorch Leaky with reset-by-subtraction):
    cur1 = x_t @ w1.T + b1
    mem1 = beta*mem1_prev + cur1 - (mem1_prev > 1)          # threshold 1.0
    spk1 = (mem1 > 1)
    cur2 = spk1 @ w2.T + b2
    mem2 = beta*mem2_prev + cur2 - (mem2_prev > 1)
    spk2 = (mem2 > 1)
Outputs: spk2 (B,T,O) and mem2 (B,T,O).

Strategy (data-parallel over batch, 16 rows per core):
  * cur1 for ALL timesteps is a feed-forward GEMM (the recurrence is only
    elementwise), computed in t-blocks of [512 x5, 384, 128, 128] columns
    (col = t*16 + b); the tapered tail keeps the final scan/GEMM2
    pipeline drain short.
  * The GEMM runs entirely in FP16 (full PE rate, half the DMA/SBUF of
    f32r) with an error-compensated 3-term split
        x@w = xh@wh + xl@wh + xh@wl,
    xh = fp16(x), xl = fp16(x - xh): 11-bit factors multiply exactly into
    the fp32 accumulator, so the scheme carries ~22 effective mantissa
    bits; measured 0/256000 spike flips vs the fp32 reference.
  * w1 is stored m-major in DRAM ((HC*128, KF*128) with row m*128+p
    holding contraction-partition p of output chunk m) so each m-block's
    weights arrive in one contiguous-per-partition DMA and block 0
    starts computing as soon as the first chunks land.
  * The layer-1 scan runs on the Vector engine with a scaled state
    M = beta*mem:
        A:  M_t = (V_{t-1} * -beta) + beta*cur_t        (scalar_tensor_tensor)
        B:  V_t = (M_t > beta) - M_t                    (scalar_tensor_tensor)
        spk_t = Sign(M_t - beta)  [Scalar engine]  stored as +-1 in f32.
    beta is folded into w1/b1/w2/b2 host-side.
  * Layer-2 currents use the sign-spike trick (spk@w2.T = s@(w2/2).T +
    rowsum(w2)/2) as a single full-precision fp32 GEMM (sign spikes are
    exact in any dtype), issued as 2 rounds of 4 column-tiled matmuls
    (M=10 output rows per 32-column PE group), so the PE runs 4 h-chunk
    matmuls concurrently.  The partition-group partials are summed by
    1 Scalar activation (+bias) and 3 Vector adds into c2.
  * The layer-2 scan is the same 2-op recurrence on (O=10, 16) tiles,
    reading c2 and writing per-block m2 tiles (c2 stays read-only, so
    no cross-engine WAR serialization).  Its ops are interleaved
    op-by-op with the layer-1 scan of the next block on the in-order
    Vector queue, so each chain's semaphore wait is hidden behind the
    other chain's execution.
  * spk2 = (m2 > beta) on Vector; mem2 = m2 * (1/beta) on Scalar; both
    stream out per block.
"""

import numpy as np

BETA = 0.95
B, T, I, H, O = 128, 200, 784, 1024, 10
NCORES = 8
BL = B // NCORES          # 16 batch rows per core
TB = T * BL               # 3200 (t-major, b-minor columns)
KF = 6                    # full 128-row contraction chunks (rows 0..767)
KT = 48                   # packed tail: [xh_t; xh_t; xl_t] x [w1h_t; w1l_t; w1h_t]
HC = H // 128             # 8 h-chunks
NBLK = (512, 512, 512, 512, 512, 384, 128, 128)
CHUNK = 512

_nc_cache = None


def _build():
    import concourse.bacc as bacc
    import concourse.mybir as mybir
    from concourse.tile import TileContext

    Alu = mybir.AluOpType
    Act = mybir.ActivationFunctionType
    f32 = mybir.dt.float32
    f16 = mybir.dt.float16

    nc = bacc.Bacc("TRN2", target_bir_lowering=False, debug=False)

    xh_d = nc.dram_tensor("xh", (KF * 128, TB), f16, kind="ExternalInput")
    xl_d = nc.dram_tensor("xl", (KF * 128, TB), f16, kind="ExternalInput")
    xt_d = nc.dram_tensor("xt", (KT, TB), f16, kind="ExternalInput")
    # m-major: row m*128+p holds w1[k-partition p, h-chunk m], 768 cols (k)
    w1h_d = nc.dram_tensor("w1h", (HC * 128, KF * 128), f16, kind="ExternalInput")
    w1l_d = nc.dram_tensor("w1l", (HC * 128, KF * 128), f16, kind="ExternalInput")
    w1t_d = nc.dram_tensor("w1t", (KT, HC * 128), f16, kind="ExternalInput")
    b1c = nc.dram_tensor("b1c", (128, HC), f32, kind="ExternalInput")
    w2p_d = nc.dram_tensor("w2p", (128, HC * O), f32, kind="ExternalInput")
    b2c = nc.dram_tensor("b2c", (O, 1), f32, kind="ExternalInput")
    S2 = nc.dram_tensor("S2", (O, TB), f32, kind="ExternalOutput")
    M2 = nc.dram_tensor("M2", (O, TB), f32, kind="ExternalOutput")

    blocks = []
    c0 = 0
    for n in NBLK:
        blocks.append((c0, n))
        c0 += n
    assert c0 == TB

    with TileContext(nc) as tc:
        with (
            tc.tile_pool(name="const", bufs=1) as cpool,
            tc.tile_pool(name="l2", bufs=1) as l2pool,
            tc.tile_pool(name="c1b", bufs=3) as c1pool,
            tc.tile_pool(name="xt", bufs=3) as xpool,
            tc.tile_pool(name="mv", bufs=2) as mvpool,
            tc.tile_pool(name="o2", bufs=2) as opool,
            tc.tile_pool(name="ps1", bufs=6, space="PSUM") as ps1,
            tc.tile_pool(name="ps2", bufs=2, space="PSUM") as ps2,
        ):
            w1h_sb = cpool.tile([128, HC, KF * 128], f16)
            w1l_sb = cpool.tile([128, HC, KF * 128], f16)
            w1t_sb = cpool.tile([KT, HC * 128], f16)
            b1_sb = cpool.tile([128, HC], f32)
            w2_sb = cpool.tile([128, HC * O], f32)
            b2_sb = cpool.tile([O, 1], f32)

            # Block-0 x tiles, DMA'd per k-chunk interleaved with the
            # per-m weight chunks so m_block(m) finds its inputs resident.
            xh0 = xpool.tile([128, KF, CHUNK], f16, tag="xh", name="xh0")
            xl0 = xpool.tile([128, KF, CHUNK], f16, tag="xl", name="xl0")
            xt0 = xpool.tile([KT, CHUNK], f16, tag="xt", name="xt0")
            n0 = blocks[0][1]
            # smallest-possible gate for the first matmul: the k0 slice
            # of w1h chunk m=0, then block-0 x k0.
            nc.sync.dma_start(out=w1h_sb[:, 0, 0:128], in_=w1h_d[0:128, 0:128])
            for k in range(KF):
                nc.sync.dma_start(
                    out=xh0[:, k, :n0], in_=xh_d[k * 128:(k + 1) * 128, 0:n0]
                )
                if k == 0:
                    nc.sync.dma_start(
                        out=w1h_sb[:, 0, 128:], in_=w1h_d[0:128, 128:]
                    )
                else:
                    nc.sync.dma_start(
                        out=w1h_sb[:, k], in_=w1h_d[k * 128:(k + 1) * 128]
                    )
                nc.sync.dma_start(
                    out=xl0[:, k, :n0], in_=xl_d[k * 128:(k + 1) * 128, 0:n0]
                )
                nc.sync.dma_start(out=w1l_sb[:, k], in_=w1l_d[k * 128:(k + 1) * 128])
            nc.sync.dma_start(out=b1_sb[:], in_=b1c[:])
            nc.sync.dma_start(out=xt0[:, :n0], in_=xt_d[:, 0:n0])
            nc.sync.dma_start(out=w1t_sb[:], in_=w1t_d[:])
            for m in range(KF, HC):
                nc.sync.dma_start(out=w1h_sb[:, m], in_=w1h_d[m * 128:(m + 1) * 128])
                nc.sync.dma_start(out=w1l_sb[:, m], in_=w1l_d[m * 128:(m + 1) * 128])
            nc.sync.dma_start(out=w2_sb[:], in_=w2p_d[:])
            nc.sync.dma_start(out=b2_sb[:], in_=b2c[:])

            c2 = l2pool.tile([O, TB], f32)       # beta*cur2 (read-only after combine)

            negbeta = cpool.tile([128, 1], f32)
            nc.vector.memset(negbeta[:], -BETA)

            v1 = mvpool.tile([128, HC * BL], f32, tag="v1")
            nc.vector.memset(v1[:], 0.0)
            v2 = mvpool.tile([O, BL], f32, tag="v2")
            nc.vector.memset(v2[:], 0.0)

            c1_tiles = {}
            spk_tiles = {}
            m2_tiles = {}

            def gemm1(bi):
                c0, n = blocks[bi]
                nt = n // BL
                if bi == 0:
                    xh, xl, xt = xh0, xl0, xt0
                else:
                    xh = xpool.tile([128, KF, CHUNK], f16, tag="xh")
                    xl = xpool.tile([128, KF, CHUNK], f16, tag="xl")
                    xt = xpool.tile([KT, CHUNK], f16, tag="xt")
                    for k in range(KF):
                        nc.sync.dma_start(
                            out=xh[:, k, :n],
                            in_=xh_d[k * 128:(k + 1) * 128, c0:c0 + n],
                        )
                        nc.sync.dma_start(
                            out=xl[:, k, :n],
                            in_=xl_d[k * 128:(k + 1) * 128, c0:c0 + n],
                        )
                    nc.sync.dma_start(out=xt[:, :n], in_=xt_d[:, c0:c0 + n])
                c1 = c1pool.tile([128, 32, HC, BL], f32, tag="c1")
                c1_tiles[bi] = c1
                spk = c1pool.tile([128, HC, 32, BL], f32, tag="spk", name="spk")
                spk_tiles[bi] = spk
                for m in range(HC):
                    p1 = ps1.tile([128, CHUNK], f32, tag="p1")
                    i = 0
                    for k in range(KF):
                        for (wt, xs_) in (
                            (w1h_sb, xh), (w1h_sb, xl), (w1l_sb, xh),
                        ):
                            nc.tensor.matmul(
                                p1[:, :n],
                                lhsT=wt[:, m, k * 128:(k + 1) * 128],
                                rhs=xs_[:, k, :n],
                                start=(i == 0),
                                stop=False,
                            )
                            i += 1
                    nc.tensor.matmul(
                        p1[:, :n],
                        lhsT=w1t_sb[:, m * 128:(m + 1) * 128],
                        rhs=xt[:, :n],
                        start=False,
                        stop=True,
                    )
                    p1v = p1.rearrange("p (t b) -> p t b", b=BL)
                    nc.scalar.activation(
                        out=c1[:, :nt, m, :],
                        in_=p1v[:, :nt, :],
                        func=Act.Identity,
                        bias=b1_sb[:, m:m + 1],
                        scale=1.0,
                    )

            def gemm2(bi):
                c0, n = blocks[bi]
                c1_tiles.pop(bi)
                spk = spk_tiles.pop(bi)
                spk2d = spk.rearrange("p c t b -> p (c t b)")
                # For the tail blocks, issue GEMM2 in two time-halves so
                # the first half only waits on half the block's scan1
                # steps (shortens the layer-2 dependency cascade).
                # Tail blocks: time-halved issue, and only 2 column
                # groups so the Vector-engine combine shrinks from 3
                # adds to 1 (the extra PE rounds land in endgame gaps).
                tail = bi >= 5
                halves = ((0, n // 2), (n // 2, n - n // 2)) if tail else ((0, n),)
                ngrp = 2 if tail else 4
                for h0, hn in halves:
                    p2 = ps2.tile([128, CHUNK], f32, tag="p2")
                    nr = HC // ngrp
                    for r in range(nr):
                        for j in range(ngrp):
                            c = r * ngrp + j
                            nc.tensor.matmul(
                                p2[32 * j:32 * j + O, :hn],
                                lhsT=w2_sb[:, c * O:(c + 1) * O],
                                rhs=spk2d[:, c * 32 * BL + h0:
                                          c * 32 * BL + h0 + hn],
                                start=(r == 0),
                                stop=(r == nr - 1),
                                tile_position=(0, 32 * j),
                            )
                    # combine the partition-group partials + bias into c2
                    nc.scalar.activation(
                        out=c2[:, c0 + h0:c0 + h0 + hn],
                        in_=p2[0:O, :hn],
                        func=Act.Identity,
                        bias=b2_sb[:, 0:1],
                        scale=1.0,
                    )
                    for j in range(1, ngrp):
                        nc.vector.tensor_tensor(
                            out=c2[:, c0 + h0:c0 + h0 + hn],
                            in0=p2[32 * j:32 * j + O, :hn],
                            in1=c2[:, c0 + h0:c0 + h0 + hn],
                            op=Alu.add,
                        )
                m2_tiles[bi] = opool.tile([O, CHUNK], f32, tag="m2", name="m2")

            def scan1_A(bi, tl):
                c1 = c1_tiles[bi]
                csf = c1[:, tl].rearrange("p c b -> p (c b)")
                m1 = mvpool.tile([128, HC * BL], f32, tag="m1")
                nc.vector.scalar_tensor_tensor(
                    out=m1[:], in0=v1[:], scalar=-BETA, in1=csf,
                    op0=Alu.mult, op1=Alu.add,
                )
                return m1

            def scan1_B(bi, tl, m1):
                nonlocal v1
                spk = spk_tiles[bi]
                v1n = mvpool.tile([128, HC * BL], f32, tag="v1")
                nc.vector.scalar_tensor_tensor(
                    out=v1n[:], in0=m1[:], scalar=BETA, in1=m1[:],
                    op0=Alu.is_gt, op1=Alu.subtract,
                )
                # sign-spikes s = 2*spk-1 on the Scalar engine; the
                # (s+1)/2 un-mapping is folded into w2/2 + bias rowsum.
                nc.scalar.activation(
                    spk[:, :, tl, :],
                    m1.rearrange("p (c b) -> p c b", b=BL),
                    Act.Sign,
                    bias=negbeta[:, 0:1], scale=1.0,
                )
                v1 = v1n

            def scan2_A(bi, tl):
                c0, n = blocks[bi]
                m2 = m2_tiles[bi]
                ms = m2[:, tl * BL:(tl + 1) * BL]
                nc.vector.scalar_tensor_tensor(
                    out=ms, in0=v2[:], scalar=-BETA,
                    in1=c2[:, c0 + tl * BL:c0 + (tl + 1) * BL],
                    op0=Alu.mult, op1=Alu.add,
                )
                return ms

            def scan2_B(ms):
                nonlocal v2
                v2n = mvpool.tile([O, BL], f32, tag="v2")
                nc.vector.scalar_tensor_tensor(
                    out=v2n[:], in0=ms, scalar=BETA, in1=ms,
                    op0=Alu.is_gt, op1=Alu.subtract,
                )
                v2 = v2n

            def scans(bi):
                """scan1 of block bi interleaved op-by-op with scan2 of
                block bi-1, emitted 1A,2A,1B,2B so every dependent pair
                on the in-order Vector queue is separated by an
                independent op (no exposed chain waits)."""
                nt1 = blocks[bi][1] // BL if bi < len(blocks) else 0
                nt2 = blocks[bi - 1][1] // BL if 1 <= bi <= len(blocks) else 0
                for tl in range(max(nt1, nt2)):
                    m1 = scan1_A(bi, tl) if tl < nt1 else None
                    ms = scan2_A(bi - 1, tl) if tl < nt2 else None
                    if m1 is not None:
                        scan1_B(bi, tl, m1)
                    if ms is not None:
                        scan2_B(ms)

            def scan2_tail_split(bi):
                """Final block's layer-2 scan as two independent
                half-batch chains, interleaved P_A,Q_A,P_B,Q_B so the
                post-GEMM drain runs with hidden chain waits."""
                nonlocal v2
                c0, n = blocks[bi]
                m2 = m2_tiles[bi]
                HB = BL // 2
                vp = v2[:, 0:HB]
                vq = v2[:, HB:BL]
                for tl in range(n // BL):
                    mp = m2[:, tl * BL:tl * BL + HB]
                    mq = m2[:, tl * BL + HB:(tl + 1) * BL]
                    nc.vector.scalar_tensor_tensor(
                        out=mp, in0=vp, scalar=-BETA,
                        in1=c2[:, c0 + tl * BL:c0 + tl * BL + HB],
                        op0=Alu.mult, op1=Alu.add,
                    )
                    nc.vector.scalar_tensor_tensor(
                        out=mq, in0=vq, scalar=-BETA,
                        in1=c2[:, c0 + tl * BL + HB:c0 + (tl + 1) * BL],
                        op0=Alu.mult, op1=Alu.add,
                    )
                    vpn = mvpool.tile([O, HB], f32, tag="vp")
                    nc.vector.scalar_tensor_tensor(
                        out=vpn[:], in0=mp, scalar=BETA, in1=mp,
                        op0=Alu.is_gt, op1=Alu.subtract,
                    )
                    vqn = mvpool.tile([O, HB], f32, tag="vq")
                    nc.vector.scalar_tensor_tensor(
                        out=vqn[:], in0=mq, scalar=BETA, in1=mq,
                        op0=Alu.is_gt, op1=Alu.subtract,
                    )
                    vp = vpn[:]
                    vq = vqn[:]

            def out2(bi):
                """spk2 (Vector) + 1/beta un-scale (Scalar) for block bi."""
                c0, n = blocks[bi]
                m2 = m2_tiles.pop(bi)
                s2b = opool.tile([O, CHUNK], f32, tag="s2b")
                nc.vector.tensor_scalar(
                    s2b[:, :n], m2[:, :n], BETA, None, Alu.is_gt,
                )
                nc.sync.dma_start(out=S2[:, c0:c0 + n], in_=s2b[:, :n])
                m2s = opool.tile([O, CHUNK], f32, tag="m2s")
                nc.scalar.activation(
                    out=m2s[:, :n], in_=m2[:, :n],
                    func=Act.Identity, bias=0.0, scale=1.0 / BETA,
                )
                nc.sync.dma_start(out=M2[:, c0:c0 + n], in_=m2s[:, :n])

            nb = len(blocks)
            for bi in range(nb):
                gemm1(bi)
                if bi > 0:
                    gemm2(bi - 1)
                scans(bi)            # scan1(bi) + scan2(bi-1) interleaved
                if bi > 1:
                    out2(bi - 2)
            gemm2(nb - 1)
            scan2_tail_split(nb - 1)
            out2(nb - 2)
            out2(nb - 1)

    nc.compile()
    return nc


def _get_nc():
    global _nc_cache
    if _nc_cache is None:
        _nc_cache = _build()
    return _nc_cache


def _split16(a):
    hi = np.asarray(a, np.float16)
    lo = np.asarray(a - hi.astype(np.float32), np.float16)
    return hi, lo


def _prep_shared(w1, b1, w2, b2):
    w1s = (BETA * w1).T.astype(np.float32)        # (784, 1024)
    w1h_f, w1l_f = _split16(w1s)

    # m-major weight layout: row m*128+p, col k*128+c = w1s[k*128+p, m*128+c]
    def mmajor(wf):
        return np.ascontiguousarray(
            wf[:768].reshape(KF, 128, HC, 128)      # (k, p, m, c)
            .transpose(2, 1, 0, 3)                  # (m, p, k, c)
            .reshape(HC * 128, KF * 128)
        )
    w1h = mmajor(w1h_f)
    w1l = mmajor(w1l_f)
    # packed 48-row tail: rows pair as (w1h,xh), (w1l,xh), (w1h,xl)
    w1t = np.ascontiguousarray(
        np.concatenate([w1h_f[768:], w1l_f[768:], w1h_f[768:]], axis=0)
    )
    b1v = np.ascontiguousarray((BETA * b1).astype(np.float32).reshape(HC, 128).T)
    # GEMM2 consumes sign-spikes s = 2*spk-1:  spk@w2.T = s@(w2/2).T + rowsum(w2)/2
    w2s = (0.5 * BETA * w2).T.astype(np.float32).reshape(HC, 128, O).transpose(1, 0, 2)
    w2p = np.ascontiguousarray(w2s.reshape(128, HC * O))
    b2v = (BETA * (b2 + 0.5 * w2.sum(axis=1))).astype(np.float32).reshape(O, 1)
    return w1h, w1l, w1t, b1v, w2p, b2v


def _make_in_maps(x, w1, b1, w2, b2):
    w1h, w1l, w1t, b1v, w2p, b2v = _prep_shared(w1, b1, w2, b2)
    in_maps = []
    for c in range(NCORES):
        xs = x[c * BL:(c + 1) * BL]                     # (BL, T, I)
        xT = np.ascontiguousarray(
            xs.transpose(2, 1, 0).reshape(I, TB)        # col = t*BL + b
        )
        xh_f, xl_f = _split16(xT)
        xh = np.ascontiguousarray(xh_f[:768])
        xl = np.ascontiguousarray(xl_f[:768])
        xt = np.ascontiguousarray(
            np.concatenate([xh_f[768:], xh_f[768:], xl_f[768:]], axis=0)
        )
        in_maps.append({
            "xh": xh, "xl": xl, "xt": xt, "w1h": w1h, "w1l": w1l, "w1t": w1t,
            "b1c": b1v, "w2p": w2p, "b2c": b2v,
        })
    return in_maps


def kernel(x, w1, b1, w2, b2):
    from concourse.bass_utils import run_bass_kernel_spmd

    nc = _get_nc()
    in_maps = _make_in_maps(x, w1, b1, w2, b2)
    res = run_bass_kernel_spmd(nc, in_maps, core_ids=list(range(NCORES)))

    spk = np.empty((B, T, O), np.float32)
    mem = np.empty((B, T, O), np.float32)
    for c in range(NCORES):
        r = res.results[c]
        spk[c * BL:(c + 1) * BL] = r["S2"].reshape(O, T, BL).transpose(2, 1, 0)
        mem[c * BL:(c + 1) * BL] = r["M2"].reshape(O, T, BL).transpose(2, 1, 0)
    return spk, mem


# revision 34
# speedup vs baseline: 1.0041x; 1.0041x over previous
"""Trainium2 Bass kernel for a 2-layer leaky-integrate-and-fire SNN.

Model (per timestep t, snnTorch Leaky with reset-by-subtraction):
    cur1 = x_t @ w1.T + b1
    mem1 = beta*mem1_prev + cur1 - (mem1_prev > 1)          # threshold 1.0
    spk1 = (mem1 > 1)
    cur2 = spk1 @ w2.T + b2
    mem2 = beta*mem2_prev + cur2 - (mem2_prev > 1)
    spk2 = (mem2 > 1)
Outputs: spk2 (B,T,O) and mem2 (B,T,O).

Strategy (data-parallel over batch, 16 rows per core):
  * cur1 for ALL timesteps is a feed-forward GEMM (the recurrence is only
    elementwise), computed in t-blocks of [512 x5, 384, 128, 128] columns
    (col = t*16 + b); the tapered tail keeps the final scan/GEMM2
    pipeline drain short.
  * The GEMM runs entirely in FP16 (full PE rate, half the DMA/SBUF of
    f32r) with an error-compensated 3-term split
        x@w = xh@wh + xl@wh + xh@wl,
    xh = fp16(x), xl = fp16(x - xh): 11-bit factors multiply exactly into
    the fp32 accumulator, so the scheme carries ~22 effective mantissa
    bits; measured 0/256000 spike flips vs the fp32 reference.
  * w1 is stored m-major in DRAM ((HC*128, KF*128) with row m*128+p
    holding contraction-partition p of output chunk m) so each m-block's
    weights arrive in one contiguous-per-partition DMA and block 0
    starts computing as soon as the first chunks land.
  * The layer-1 scan runs on the Vector engine with a scaled state
    M = beta*mem:
        A:  M_t = (V_{t-1} * -beta) + beta*cur_t        (scalar_tensor_tensor)
        B:  V_t = (M_t > beta) - M_t                    (scalar_tensor_tensor)
        spk_t = Sign(M_t - beta)  [Scalar engine]  stored as +-1 in f32.
    beta is folded into w1/b1/w2/b2 host-side.
  * Layer-2 currents use the sign-spike trick (spk@w2.T = s@(w2/2).T +
    rowsum(w2)/2) as a single full-precision fp32 GEMM (sign spikes are
    exact in any dtype), issued as 2 rounds of 4 column-tiled matmuls
    (M=10 output rows per 32-column PE group), so the PE runs 4 h-chunk
    matmuls concurrently.  The partition-group partials are summed by
    1 Scalar activation (+bias) and 3 Vector adds into c2.
  * The layer-2 scan is the same 2-op recurrence on (O=10, 16) tiles,
    reading c2 and writing per-block m2 tiles (c2 stays read-only, so
    no cross-engine WAR serialization).  Its ops are interleaved
    op-by-op with the layer-1 scan of the next block on the in-order
    Vector queue, so each chain's semaphore wait is hidden behind the
    other chain's execution.
  * spk2 = (m2 > beta) on Vector; mem2 = m2 * (1/beta) on Scalar; both
    stream out per block.
"""

import numpy as np

BETA = 0.95
B, T, I, H, O = 128, 200, 784, 1024, 10
NCORES = 8
BL = B // NCORES          # 16 batch rows per core
TB = T * BL               # 3200 (t-major, b-minor columns)
KF = 6                    # full 128-row contraction chunks (rows 0..767)
KT = 48                   # packed tail: [xh_t; xh_t; xl_t] x [w1h_t; w1l_t; w1h_t]
HC = H // 128             # 8 h-chunks
NBLK = (512, 512, 512, 512, 512, 384, 128, 128)
CHUNK = 512

_nc_cache = None


def _build():
    import concourse.bacc as bacc
    import concourse.mybir as mybir
    from concourse.tile import TileContext

    Alu = mybir.AluOpType
    Act = mybir.ActivationFunctionType
    f32 = mybir.dt.float32
    f16 = mybir.dt.float16

    nc = bacc.Bacc("TRN2", target_bir_lowering=False, debug=False)

    xh_d = nc.dram_tensor("xh", (KF * 128, TB), f16, kind="ExternalInput")
    xl_d = nc.dram_tensor("xl", (KF * 128, TB), f16, kind="ExternalInput")
    xt_d = nc.dram_tensor("xt", (KT, TB), f16, kind="ExternalInput")
    # m-major: row m*128+p holds w1[k-partition p, h-chunk m], 768 cols (k)
    w1h_d = nc.dram_tensor("w1h", (HC * 128, KF * 128), f16, kind="ExternalInput")
    w1l_d = nc.dram_tensor("w1l", (HC * 128, KF * 128), f16, kind="ExternalInput")
    w1t_d = nc.dram_tensor("w1t", (KT, HC * 128), f16, kind="ExternalInput")
    b1c = nc.dram_tensor("b1c", (128, HC), f32, kind="ExternalInput")
    w2p_d = nc.dram_tensor("w2p", (128, HC * O), f32, kind="ExternalInput")
    b2c = nc.dram_tensor("b2c", (O, 1), f32, kind="ExternalInput")
    S2 = nc.dram_tensor("S2", (O, TB), f32, kind="ExternalOutput")
    M2 = nc.dram_tensor("M2", (O, TB), f32, kind="ExternalOutput")

    blocks = []
    c0 = 0
    for n in NBLK:
        blocks.append((c0, n))
        c0 += n
    assert c0 == TB

    with TileContext(nc) as tc:
        with (
            tc.tile_pool(name="const", bufs=1) as cpool,
            tc.tile_pool(name="l2", bufs=1) as l2pool,
            tc.tile_pool(name="c1b", bufs=3) as c1pool,
            tc.tile_pool(name="xt", bufs=3) as xpool,
            tc.tile_pool(name="mv", bufs=2) as mvpool,
            tc.tile_pool(name="o2", bufs=2) as opool,
            tc.tile_pool(name="ps1", bufs=6, space="PSUM") as ps1,
            tc.tile_pool(name="ps2", bufs=2, space="PSUM") as ps2,
        ):
            w1h_sb = cpool.tile([128, HC, KF * 128], f16)
            w1l_sb = cpool.tile([128, HC, KF * 128], f16)
            w1t_sb = cpool.tile([KT, HC * 128], f16)
            b1_sb = cpool.tile([128, HC], f32)
            w2_sb = cpool.tile([128, HC * O], f32)
            b2_sb = cpool.tile([O, 1], f32)

            # Block-0 x tiles, DMA'd per k-chunk interleaved with the
            # per-m weight chunks so m_block(m) finds its inputs resident.
            xh0 = xpool.tile([128, KF, CHUNK], f16, tag="xh", name="xh0")
            xl0 = xpool.tile([128, KF, CHUNK], f16, tag="xl", name="xl0")
            xt0 = xpool.tile([KT, CHUNK], f16, tag="xt", name="xt0")
            n0 = blocks[0][1]
            # smallest-possible gate for the first matmul: the k0 slice
            # of w1h chunk m=0, then block-0 x k0.
            nc.sync.dma_start(out=w1h_sb[:, 0, 0:128], in_=w1h_d[0:128, 0:128])
            for k in range(KF):
                nc.sync.dma_start(
                    out=xh0[:, k, :n0], in_=xh_d[k * 128:(k + 1) * 128, 0:n0]
                )
                if k == 0:
                    nc.sync.dma_start(
                        out=w1h_sb[:, 0, 128:], in_=w1h_d[0:128, 128:]
                    )
                else:
                    nc.sync.dma_start(
                        out=w1h_sb[:, k], in_=w1h_d[k * 128:(k + 1) * 128]
                    )
                nc.sync.dma_start(
                    out=xl0[:, k, :n0], in_=xl_d[k * 128:(k + 1) * 128, 0:n0]
                )
                nc.sync.dma_start(out=w1l_sb[:, k], in_=w1l_d[k * 128:(k + 1) * 128])
            nc.sync.dma_start(out=b1_sb[:], in_=b1c[:])
            nc.sync.dma_start(out=xt0[:, :n0], in_=xt_d[:, 0:n0])
            nc.sync.dma_start(out=w1t_sb[:], in_=w1t_d[:])
            for m in range(KF, HC):
                nc.sync.dma_start(out=w1h_sb[:, m], in_=w1h_d[m * 128:(m + 1) * 128])
                nc.sync.dma_start(out=w1l_sb[:, m], in_=w1l_d[m * 128:(m + 1) * 128])
            nc.sync.dma_start(out=w2_sb[:], in_=w2p_d[:])
            nc.sync.dma_start(out=b2_sb[:], in_=b2c[:])

            c2 = l2pool.tile([O, TB], f32)       # beta*cur2 (read-only after combine)

            negbeta = cpool.tile([128, 1], f32)
            nc.vector.memset(negbeta[:], -BETA)

            v1 = mvpool.tile([128, HC * BL], f32, tag="v1")
            nc.vector.memset(v1[:], 0.0)
            v2 = mvpool.tile([O, BL], f32, tag="v2")
            nc.vector.memset(v2[:], 0.0)

            c1_tiles = {}
            spk_tiles = {}
            m2_tiles = {}

            def gemm1(bi):
                c0, n = blocks[bi]
                nt = n // BL
                if bi == 0:
                    xh, xl, xt = xh0, xl0, xt0
                else:
                    xh = xpool.tile([128, KF, CHUNK], f16, tag="xh")
                    xl = xpool.tile([128, KF, CHUNK], f16, tag="xl")
                    xt = xpool.tile([KT, CHUNK], f16, tag="xt")
                    for k in range(KF):
                        nc.sync.dma_start(
                            out=xh[:, k, :n],
                            in_=xh_d[k * 128:(k + 1) * 128, c0:c0 + n],
                        )
                        nc.sync.dma_start(
                            out=xl[:, k, :n],
                            in_=xl_d[k * 128:(k + 1) * 128, c0:c0 + n],
                        )
                    nc.sync.dma_start(out=xt[:, :n], in_=xt_d[:, c0:c0 + n])
                c1 = c1pool.tile([128, 32, HC, BL], f32, tag="c1")
                c1_tiles[bi] = c1
                spk = c1pool.tile([128, HC, 32, BL], f32, tag="spk", name="spk")
                spk_tiles[bi] = spk
                for m in range(HC):
                    p1 = ps1.tile([128, CHUNK], f32, tag="p1")
                    i = 0
                    for k in range(KF):
                        for (wt, xs_) in (
                            (w1h_sb, xh), (w1h_sb, xl), (w1l_sb, xh),
                        ):
                            nc.tensor.matmul(
                                p1[:, :n],
                                lhsT=wt[:, m, k * 128:(k + 1) * 128],
                                rhs=xs_[:, k, :n],
                                start=(i == 0),
                                stop=False,
                            )
                            i += 1
                    nc.tensor.matmul(
                        p1[:, :n],
                        lhsT=w1t_sb[:, m * 128:(m + 1) * 128],
                        rhs=xt[:, :n],
                        start=False,
                        stop=True,
                    )
                    p1v = p1.rearrange("p (t b) -> p t b", b=BL)
                    nc.scalar.activation(
                        out=c1[:, :nt, m, :],
                        in_=p1v[:, :nt, :],
                        func=Act.Identity,
                        bias=b1_sb[:, m:m + 1],
                        scale=1.0,
                    )

            def gemm2(bi):
                c0, n = blocks[bi]
                c1_tiles.pop(bi)
                spk = spk_tiles.pop(bi)
                spk2d = spk.rearrange("p c t b -> p (c t b)")
                # For the tail blocks, issue GEMM2 in two time-halves so
                # the first half only waits on half the block's scan1
                # steps (shortens the layer-2 dependency cascade).
                # Tail blocks: time-halved issue, and only 2 column
                # groups so the Vector-engine combine shrinks from 3
                # adds to 1 (the extra PE rounds land in endgame gaps).
                tail = bi >= 5
                halves = ((0, n // 2), (n // 2, n - n // 2)) if tail else ((0, n),)
                ngrp = 2 if bi >= 4 else 4
                for h0, hn in halves:
                    p2 = ps2.tile([128, CHUNK], f32, tag="p2")
                    nr = HC // ngrp
                    for r in range(nr):
                        for j in range(ngrp):
                            c = r * ngrp + j
                            nc.tensor.matmul(
                                p2[32 * j:32 * j + O, :hn],
                                lhsT=w2_sb[:, c * O:(c + 1) * O],
                                rhs=spk2d[:, c * 32 * BL + h0:
                                          c * 32 * BL + h0 + hn],
                                start=(r == 0),
                                stop=(r == nr - 1),
                                tile_position=(0, 32 * j),
                            )
                    # combine the partition-group partials + bias into c2
                    nc.scalar.activation(
                        out=c2[:, c0 + h0:c0 + h0 + hn],
                        in_=p2[0:O, :hn],
                        func=Act.Identity,
                        bias=b2_sb[:, 0:1],
                        scale=1.0,
                    )
                    for j in range(1, ngrp):
                        nc.vector.tensor_tensor(
                            out=c2[:, c0 + h0:c0 + h0 + hn],
                            in0=p2[32 * j:32 * j + O, :hn],
                            in1=c2[:, c0 + h0:c0 + h0 + hn],
                            op=Alu.add,
                        )
                m2_tiles[bi] = opool.tile([O, CHUNK], f32, tag="m2", name="m2")

            def scan1_A(bi, tl):
                c1 = c1_tiles[bi]
                csf = c1[:, tl].rearrange("p c b -> p (c b)")
                m1 = mvpool.tile([128, HC * BL], f32, tag="m1")
                nc.vector.scalar_tensor_tensor(
                    out=m1[:], in0=v1[:], scalar=-BETA, in1=csf,
                    op0=Alu.mult, op1=Alu.add,
                )
                return m1

            def scan1_B(bi, tl, m1):
                nonlocal v1
                spk = spk_tiles[bi]
                v1n = mvpool.tile([128, HC * BL], f32, tag="v1")
                nc.vector.scalar_tensor_tensor(
                    out=v1n[:], in0=m1[:], scalar=BETA, in1=m1[:],
                    op0=Alu.is_gt, op1=Alu.subtract,
                )
                # sign-spikes s = 2*spk-1 on the Scalar engine; the
                # (s+1)/2 un-mapping is folded into w2/2 + bias rowsum.
                nc.scalar.activation(
                    spk[:, :, tl, :],
                    m1.rearrange("p (c b) -> p c b", b=BL),
                    Act.Sign,
                    bias=negbeta[:, 0:1], scale=1.0,
                )
                v1 = v1n

            def scan2_A(bi, tl):
                c0, n = blocks[bi]
                m2 = m2_tiles[bi]
                ms = m2[:, tl * BL:(tl + 1) * BL]
                nc.vector.scalar_tensor_tensor(
                    out=ms, in0=v2[:], scalar=-BETA,
                    in1=c2[:, c0 + tl * BL:c0 + (tl + 1) * BL],
                    op0=Alu.mult, op1=Alu.add,
                )
                return ms

            def scan2_B(ms):
                nonlocal v2
                v2n = mvpool.tile([O, BL], f32, tag="v2")
                nc.vector.scalar_tensor_tensor(
                    out=v2n[:], in0=ms, scalar=BETA, in1=ms,
                    op0=Alu.is_gt, op1=Alu.subtract,
                )
                v2 = v2n

            def scans(bi):
                """scan1 of block bi interleaved op-by-op with scan2 of
                block bi-1, emitted 1A,2A,1B,2B so every dependent pair
                on the in-order Vector queue is separated by an
                independent op (no exposed chain waits)."""
                nt1 = blocks[bi][1] // BL if bi < len(blocks) else 0
                nt2 = blocks[bi - 1][1] // BL if 1 <= bi <= len(blocks) else 0
                for tl in range(max(nt1, nt2)):
                    m1 = scan1_A(bi, tl) if tl < nt1 else None
                    ms = scan2_A(bi - 1, tl) if tl < nt2 else None
                    if m1 is not None:
                        scan1_B(bi, tl, m1)
                    if ms is not None:
                        scan2_B(ms)

            def scan2_tail_split(bi):
                """Final block's layer-2 scan as two independent
                half-batch chains, interleaved P_A,Q_A,P_B,Q_B so the
                post-GEMM drain runs with hidden chain waits."""
                nonlocal v2
                c0, n = blocks[bi]
                m2 = m2_tiles[bi]
                HB = BL // 2
                vp = v2[:, 0:HB]
                vq = v2[:, HB:BL]
                for tl in range(n // BL):
                    mp = m2[:, tl * BL:tl * BL + HB]
                    mq = m2[:, tl * BL + HB:(tl + 1) * BL]
                    nc.vector.scalar_tensor_tensor(
                        out=mp, in0=vp, scalar=-BETA,
                        in1=c2[:, c0 + tl * BL:c0 + tl * BL + HB],
                        op0=Alu.mult, op1=Alu.add,
                    )
                    nc.vector.scalar_tensor_tensor(
                        out=mq, in0=vq, scalar=-BETA,
                        in1=c2[:, c0 + tl * BL + HB:c0 + (tl + 1) * BL],
                        op0=Alu.mult, op1=Alu.add,
                    )
                    vpn = mvpool.tile([O, HB], f32, tag="vp")
                    nc.vector.scalar_tensor_tensor(
                        out=vpn[:], in0=mp, scalar=BETA, in1=mp,
                        op0=Alu.is_gt, op1=Alu.subtract,
                    )
                    vqn = mvpool.tile([O, HB], f32, tag="vq")
                    nc.vector.scalar_tensor_tensor(
                        out=vqn[:], in0=mq, scalar=BETA, in1=mq,
                        op0=Alu.is_gt, op1=Alu.subtract,
                    )
                    vp = vpn[:]
                    vq = vqn[:]

            def out2(bi):
                """spk2 (Vector) + 1/beta un-scale (Scalar) for block bi."""
                c0, n = blocks[bi]
                m2 = m2_tiles.pop(bi)
                s2b = opool.tile([O, CHUNK], f32, tag="s2b")
                nc.vector.tensor_scalar(
                    s2b[:, :n], m2[:, :n], BETA, None, Alu.is_gt,
                )
                nc.sync.dma_start(out=S2[:, c0:c0 + n], in_=s2b[:, :n])
                m2s = opool.tile([O, CHUNK], f32, tag="m2s")
                nc.scalar.activation(
                    out=m2s[:, :n], in_=m2[:, :n],
                    func=Act.Identity, bias=0.0, scale=1.0 / BETA,
                )
                nc.sync.dma_start(out=M2[:, c0:c0 + n], in_=m2s[:, :n])

            nb = len(blocks)
            for bi in range(nb):
                gemm1(bi)
                if bi > 0:
                    gemm2(bi - 1)
                scans(bi)            # scan1(bi) + scan2(bi-1) interleaved
                if bi > 1:
                    out2(bi - 2)
            gemm2(nb - 1)
            scan2_tail_split(nb - 1)
            out2(nb - 2)
            out2(nb - 1)

    nc.compile()
    return nc


def _get_nc():
    global _nc_cache
    if _nc_cache is None:
        _nc_cache = _build()
    return _nc_cache


def _split16(a):
    hi = np.asarray(a, np.float16)
    lo = np.asarray(a - hi.astype(np.float32), np.float16)
    return hi, lo


def _prep_shared(w1, b1, w2, b2):
    w1s = (BETA * w1).T.astype(np.float32)        # (784, 1024)
    w1h_f, w1l_f = _split16(w1s)

    # m-major weight layout: row m*128+p, col k*128+c = w1s[k*128+p, m*128+c]
    def mmajor(wf):
        return np.ascontiguousarray(
            wf[:768].reshape(KF, 128, HC, 128)      # (k, p, m, c)
            .transpose(2, 1, 0, 3)                  # (m, p, k, c)
            .reshape(HC * 128, KF * 128)
        )
    w1h = mmajor(w1h_f)
    w1l = mmajor(w1l_f)
    # packed 48-row tail: rows pair as (w1h,xh), (w1l,xh), (w1h,xl)
    w1t = np.ascontiguousarray(
        np.concatenate([w1h_f[768:], w1l_f[768:], w1h_f[768:]], axis=0)
    )
    b1v = np.ascontiguousarray((BETA * b1).astype(np.float32).reshape(HC, 128).T)
    # GEMM2 consumes sign-spikes s = 2*spk-1:  spk@w2.T = s@(w2/2).T + rowsum(w2)/2
    w2s = (0.5 * BETA * w2).T.astype(np.float32).reshape(HC, 128, O).transpose(1, 0, 2)
    w2p = np.ascontiguousarray(w2s.reshape(128, HC * O))
    b2v = (BETA * (b2 + 0.5 * w2.sum(axis=1))).astype(np.float32).reshape(O, 1)
    return w1h, w1l, w1t, b1v, w2p, b2v


def _make_in_maps(x, w1, b1, w2, b2):
    w1h, w1l, w1t, b1v, w2p, b2v = _prep_shared(w1, b1, w2, b2)
    in_maps = []
    for c in range(NCORES):
        xs = x[c * BL:(c + 1) * BL]                     # (BL, T, I)
        xT = np.ascontiguousarray(
            xs.transpose(2, 1, 0).reshape(I, TB)        # col = t*BL + b
        )
        xh_f, xl_f = _split16(xT)
        xh = np.ascontiguousarray(xh_f[:768])
        xl = np.ascontiguousarray(xl_f[:768])
        xt = np.ascontiguousarray(
            np.concatenate([xh_f[768:], xh_f[768:], xl_f[768:]], axis=0)
        )
        in_maps.append({
            "xh": xh, "xl": xl, "xt": xt, "w1h": w1h, "w1l": w1l, "w1t": w1t,
            "b1c": b1v, "w2p": w2p, "b2c": b2v,
        })
    return in_maps


def kernel(x, w1, b1, w2, b2):
    from concourse.bass_utils import run_bass_kernel_spmd

    nc = _get_nc()
    in_maps = _make_in_maps(x, w1, b1, w2, b2)
    res = run_bass_kernel_spmd(nc, in_maps, core_ids=list(range(NCORES)))

    spk = np.empty((B, T, O), np.float32)
    mem = np.empty((B, T, O), np.float32)
    for c in range(NCORES):
        r = res.results[c]
        spk[c * BL:(c + 1) * BL] = r["S2"].reshape(O, T, BL).transpose(2, 1, 0)
        mem[c * BL:(c + 1) * BL] = r["M2"].reshape(O, T, BL).transpose(2, 1, 0)
    return spk, mem


# revision 35
# speedup vs baseline: 1.0057x; 1.0016x over previous
"""Trainium2 Bass kernel for a 2-layer leaky-integrate-and-fire SNN.

Model (per timestep t, snnTorch Leaky with reset-by-subtraction):
    cur1 = x_t @ w1.T + b1
    mem1 = beta*mem1_prev + cur1 - (mem1_prev > 1)          # threshold 1.0
    spk1 = (mem1 > 1)
    cur2 = spk1 @ w2.T + b2
    mem2 = beta*mem2_prev + cur2 - (mem2_prev > 1)
    spk2 = (mem2 > 1)
Outputs: spk2 (B,T,O) and mem2 (B,T,O).

Strategy (data-parallel over batch, 16 rows per core):
  * cur1 for ALL timesteps is a feed-forward GEMM (the recurrence is only
    elementwise), computed in t-blocks of [512 x5, 384, 128, 128] columns
    (col = t*16 + b); the tapered tail keeps the final scan/GEMM2
    pipeline drain short.
  * The GEMM runs entirely in FP16 (full PE rate, half the DMA/SBUF of
    f32r) with an error-compensated 3-term split
        x@w = xh@wh + xl@wh + xh@wl,
    xh = fp16(x), xl = fp16(x - xh): 11-bit factors multiply exactly into
    the fp32 accumulator, so the scheme carries ~22 effective mantissa
    bits; measured 0/256000 spike flips vs the fp32 reference.
  * w1 is stored m-major in DRAM ((HC*128, KF*128) with row m*128+p
    holding contraction-partition p of output chunk m) so each m-block's
    weights arrive in one contiguous-per-partition DMA and block 0
    starts computing as soon as the first chunks land.
  * The layer-1 scan runs on the Vector engine with a scaled state
    M = beta*mem:
        A:  M_t = (V_{t-1} * -beta) + beta*cur_t        (scalar_tensor_tensor)
        B:  V_t = (M_t > beta) - M_t                    (scalar_tensor_tensor)
        spk_t = Sign(M_t - beta)  [Scalar engine]  stored as +-1 in f32.
    beta is folded into w1/b1/w2/b2 host-side.
  * Layer-2 currents use the sign-spike trick (spk@w2.T = s@(w2/2).T +
    rowsum(w2)/2) as a single full-precision fp32 GEMM (sign spikes are
    exact in any dtype), issued as 2 rounds of 4 column-tiled matmuls
    (M=10 output rows per 32-column PE group), so the PE runs 4 h-chunk
    matmuls concurrently.  The partition-group partials are summed by
    1 Scalar activation (+bias) and 3 Vector adds into c2.
  * The layer-2 scan is the same 2-op recurrence on (O=10, 16) tiles,
    reading c2 and writing per-block m2 tiles (c2 stays read-only, so
    no cross-engine WAR serialization).  Its ops are interleaved
    op-by-op with the layer-1 scan of the next block on the in-order
    Vector queue, so each chain's semaphore wait is hidden behind the
    other chain's execution.
  * spk2 = (m2 > beta) on Vector; mem2 = m2 * (1/beta) on Scalar; both
    stream out per block.
"""

import numpy as np

BETA = 0.95
B, T, I, H, O = 128, 200, 784, 1024, 10
NCORES = 8
BL = B // NCORES          # 16 batch rows per core
TB = T * BL               # 3200 (t-major, b-minor columns)
KF = 6                    # full 128-row contraction chunks (rows 0..767)
KT = 48                   # packed tail: [xh_t; xh_t; xl_t] x [w1h_t; w1l_t; w1h_t]
HC = H // 128             # 8 h-chunks
NBLK = (512, 512, 512, 512, 512, 384, 128, 128)
CHUNK = 512

_nc_cache = None


def _build():
    import concourse.bacc as bacc
    import concourse.mybir as mybir
    from concourse.tile import TileContext

    Alu = mybir.AluOpType
    Act = mybir.ActivationFunctionType
    f32 = mybir.dt.float32
    f16 = mybir.dt.float16

    nc = bacc.Bacc("TRN2", target_bir_lowering=False, debug=False)

    xh_d = nc.dram_tensor("xh", (KF * 128, TB), f16, kind="ExternalInput")
    xl_d = nc.dram_tensor("xl", (KF * 128, TB), f16, kind="ExternalInput")
    xt_d = nc.dram_tensor("xt", (KT, TB), f16, kind="ExternalInput")
    # m-major: row m*128+p holds w1[k-partition p, h-chunk m], 768 cols (k)
    w1h_d = nc.dram_tensor("w1h", (HC * 128, KF * 128), f16, kind="ExternalInput")
    w1l_d = nc.dram_tensor("w1l", (HC * 128, KF * 128), f16, kind="ExternalInput")
    w1t_d = nc.dram_tensor("w1t", (KT, HC * 128), f16, kind="ExternalInput")
    b1c = nc.dram_tensor("b1c", (128, HC), f32, kind="ExternalInput")
    w2p_d = nc.dram_tensor("w2p", (128, HC * O), f32, kind="ExternalInput")
    b2c = nc.dram_tensor("b2c", (O, 1), f32, kind="ExternalInput")
    S2 = nc.dram_tensor("S2", (O, TB), f32, kind="ExternalOutput")
    M2 = nc.dram_tensor("M2", (O, TB), f32, kind="ExternalOutput")

    blocks = []
    c0 = 0
    for n in NBLK:
        blocks.append((c0, n))
        c0 += n
    assert c0 == TB

    with TileContext(nc) as tc:
        with (
            tc.tile_pool(name="const", bufs=1) as cpool,
            tc.tile_pool(name="l2", bufs=1) as l2pool,
            tc.tile_pool(name="c1b", bufs=3) as c1pool,
            tc.tile_pool(name="xt", bufs=3) as xpool,
            tc.tile_pool(name="mv", bufs=2) as mvpool,
            tc.tile_pool(name="o2", bufs=2) as opool,
            tc.tile_pool(name="ps1", bufs=6, space="PSUM") as ps1,
            tc.tile_pool(name="ps2", bufs=2, space="PSUM") as ps2,
        ):
            w1h_sb = cpool.tile([128, HC, KF * 128], f16)
            w1l_sb = cpool.tile([128, HC, KF * 128], f16)
            w1t_sb = cpool.tile([KT, HC * 128], f16)
            b1_sb = cpool.tile([128, HC], f32)
            w2_sb = cpool.tile([128, HC * O], f32)
            b2_sb = cpool.tile([O, 1], f32)

            # Block-0 x tiles, DMA'd per k-chunk interleaved with the
            # per-m weight chunks so m_block(m) finds its inputs resident.
            xh0 = xpool.tile([128, KF, CHUNK], f16, tag="xh", name="xh0")
            xl0 = xpool.tile([128, KF, CHUNK], f16, tag="xl", name="xl0")
            xt0 = xpool.tile([KT, CHUNK], f16, tag="xt", name="xt0")
            n0 = blocks[0][1]
            # smallest-possible gate for the first matmul: the k0 slice
            # of w1h chunk m=0, then block-0 x k0.
            nc.sync.dma_start(out=w1h_sb[:, 0, 0:128], in_=w1h_d[0:128, 0:128])
            for k in range(KF):
                nc.sync.dma_start(
                    out=xh0[:, k, :n0], in_=xh_d[k * 128:(k + 1) * 128, 0:n0]
                )
                if k == 0:
                    nc.sync.dma_start(
                        out=w1h_sb[:, 0, 128:], in_=w1h_d[0:128, 128:]
                    )
                else:
                    nc.sync.dma_start(
                        out=w1h_sb[:, k], in_=w1h_d[k * 128:(k + 1) * 128]
                    )
                nc.sync.dma_start(
                    out=xl0[:, k, :n0], in_=xl_d[k * 128:(k + 1) * 128, 0:n0]
                )
                nc.sync.dma_start(out=w1l_sb[:, k], in_=w1l_d[k * 128:(k + 1) * 128])
            nc.sync.dma_start(out=b1_sb[:], in_=b1c[:])
            nc.sync.dma_start(out=xt0[:, :n0], in_=xt_d[:, 0:n0])
            nc.sync.dma_start(out=w1t_sb[:], in_=w1t_d[:])
            for m in range(KF, HC):
                nc.sync.dma_start(out=w1h_sb[:, m], in_=w1h_d[m * 128:(m + 1) * 128])
                nc.sync.dma_start(out=w1l_sb[:, m], in_=w1l_d[m * 128:(m + 1) * 128])
            nc.sync.dma_start(out=w2_sb[:], in_=w2p_d[:])
            nc.sync.dma_start(out=b2_sb[:], in_=b2c[:])

            c2 = l2pool.tile([O, TB], f32)       # beta*cur2 (read-only after combine)

            negbeta = cpool.tile([128, 1], f32)
            nc.vector.memset(negbeta[:], -BETA)

            v1 = mvpool.tile([128, HC * BL], f32, tag="v1")
            nc.vector.memset(v1[:], 0.0)
            v2 = mvpool.tile([O, BL], f32, tag="v2")
            nc.vector.memset(v2[:], 0.0)

            c1_tiles = {}
            spk_tiles = {}
            m2_tiles = {}

            def gemm1(bi):
                c0, n = blocks[bi]
                nt = n // BL
                if bi == 0:
                    xh, xl, xt = xh0, xl0, xt0
                else:
                    xh = xpool.tile([128, KF, CHUNK], f16, tag="xh")
                    xl = xpool.tile([128, KF, CHUNK], f16, tag="xl")
                    xt = xpool.tile([KT, CHUNK], f16, tag="xt")
                    for k in range(KF):
                        nc.sync.dma_start(
                            out=xh[:, k, :n],
                            in_=xh_d[k * 128:(k + 1) * 128, c0:c0 + n],
                        )
                        nc.sync.dma_start(
                            out=xl[:, k, :n],
                            in_=xl_d[k * 128:(k + 1) * 128, c0:c0 + n],
                        )
                    nc.sync.dma_start(out=xt[:, :n], in_=xt_d[:, c0:c0 + n])
                c1 = c1pool.tile([128, 32, HC, BL], f32, tag="c1")
                c1_tiles[bi] = c1
                spk = c1pool.tile([128, HC, 32, BL], f32, tag="spk", name="spk")
                spk_tiles[bi] = spk
                for m in range(HC):
                    p1 = ps1.tile([128, CHUNK], f32, tag="p1")
                    i = 0
                    for k in range(KF):
                        for (wt, xs_) in (
                            (w1h_sb, xh), (w1h_sb, xl), (w1l_sb, xh),
                        ):
                            nc.tensor.matmul(
                                p1[:, :n],
                                lhsT=wt[:, m, k * 128:(k + 1) * 128],
                                rhs=xs_[:, k, :n],
                                start=(i == 0),
                                stop=False,
                            )
                            i += 1
                    nc.tensor.matmul(
                        p1[:, :n],
                        lhsT=w1t_sb[:, m * 128:(m + 1) * 128],
                        rhs=xt[:, :n],
                        start=False,
                        stop=True,
                    )
                    p1v = p1.rearrange("p (t b) -> p t b", b=BL)
                    nc.scalar.activation(
                        out=c1[:, :nt, m, :],
                        in_=p1v[:, :nt, :],
                        func=Act.Identity,
                        bias=b1_sb[:, m:m + 1],
                        scale=1.0,
                    )

            def gemm2(bi):
                c0, n = blocks[bi]
                c1_tiles.pop(bi)
                spk = spk_tiles.pop(bi)
                spk2d = spk.rearrange("p c t b -> p (c t b)")
                # For the tail blocks, issue GEMM2 in two time-halves so
                # the first half only waits on half the block's scan1
                # steps (shortens the layer-2 dependency cascade).
                # Tail blocks: time-halved issue, and only 2 column
                # groups so the Vector-engine combine shrinks from 3
                # adds to 1 (the extra PE rounds land in endgame gaps).
                tail = bi >= 5
                halves = ((0, n // 2), (n // 2, n - n // 2)) if tail else ((0, n),)
                ngrp = 2 if tail else 4
                for h0, hn in halves:
                    p2 = ps2.tile([128, CHUNK], f32, tag="p2")
                    nr = HC // ngrp
                    for r in range(nr):
                        for j in range(ngrp):
                            c = r * ngrp + j
                            nc.tensor.matmul(
                                p2[32 * j:32 * j + O, :hn],
                                lhsT=w2_sb[:, c * O:(c + 1) * O],
                                rhs=spk2d[:, c * 32 * BL + h0:
                                          c * 32 * BL + h0 + hn],
                                start=(r == 0),
                                stop=(r == nr - 1),
                                tile_position=(0, 32 * j),
                            )
                    # combine the partition-group partials + bias into c2
                    nc.scalar.activation(
                        out=c2[:, c0 + h0:c0 + h0 + hn],
                        in_=p2[0:O, :hn],
                        func=Act.Identity,
                        bias=b2_sb[:, 0:1],
                        scale=1.0,
                    )
                    for j in range(1, ngrp):
                        nc.vector.tensor_tensor(
                            out=c2[:, c0 + h0:c0 + h0 + hn],
                            in0=p2[32 * j:32 * j + O, :hn],
                            in1=c2[:, c0 + h0:c0 + h0 + hn],
                            op=Alu.add,
                        )
                m2_tiles[bi] = opool.tile([O, CHUNK], f32, tag="m2", name="m2")

            def scan1_A(bi, tl):
                c1 = c1_tiles[bi]
                csf = c1[:, tl].rearrange("p c b -> p (c b)")
                m1 = mvpool.tile([128, HC * BL], f32, tag="m1")
                nc.vector.scalar_tensor_tensor(
                    out=m1[:], in0=v1[:], scalar=-BETA, in1=csf,
                    op0=Alu.mult, op1=Alu.add,
                )
                return m1

            def scan1_B(bi, tl, m1):
                nonlocal v1
                spk = spk_tiles[bi]
                v1n = mvpool.tile([128, HC * BL], f32, tag="v1")
                nc.vector.scalar_tensor_tensor(
                    out=v1n[:], in0=m1[:], scalar=BETA, in1=m1[:],
                    op0=Alu.is_gt, op1=Alu.subtract,
                )
                # sign-spikes s = 2*spk-1 on the Scalar engine; the
                # (s+1)/2 un-mapping is folded into w2/2 + bias rowsum.
                nc.scalar.activation(
                    spk[:, :, tl, :],
                    m1.rearrange("p (c b) -> p c b", b=BL),
                    Act.Sign,
                    bias=negbeta[:, 0:1], scale=1.0,
                )
                v1 = v1n

            def scan2_A(bi, tl):
                c0, n = blocks[bi]
                m2 = m2_tiles[bi]
                ms = m2[:, tl * BL:(tl + 1) * BL]
                nc.vector.scalar_tensor_tensor(
                    out=ms, in0=v2[:], scalar=-BETA,
                    in1=c2[:, c0 + tl * BL:c0 + (tl + 1) * BL],
                    op0=Alu.mult, op1=Alu.add,
                )
                return ms

            def scan2_B(ms):
                nonlocal v2
                v2n = mvpool.tile([O, BL], f32, tag="v2")
                nc.vector.scalar_tensor_tensor(
                    out=v2n[:], in0=ms, scalar=BETA, in1=ms,
                    op0=Alu.is_gt, op1=Alu.subtract,
                )
                v2 = v2n

            def scans(bi):
                """scan1 of block bi interleaved op-by-op with scan2 of
                block bi-1, emitted 1A,2A,1B,2B so every dependent pair
                on the in-order Vector queue is separated by an
                independent op (no exposed chain waits)."""
                nt1 = blocks[bi][1] // BL if bi < len(blocks) else 0
                nt2 = blocks[bi - 1][1] // BL if 1 <= bi <= len(blocks) else 0
                for tl in range(max(nt1, nt2)):
                    m1 = scan1_A(bi, tl) if tl < nt1 else None
                    ms = scan2_A(bi - 1, tl) if tl < nt2 else None
                    if m1 is not None:
                        scan1_B(bi, tl, m1)
                    if ms is not None:
                        scan2_B(ms)

            def scan2_tail_split(bi):
                """Final block's layer-2 scan as two independent
                half-batch chains, interleaved P_A,Q_A,P_B,Q_B so the
                post-GEMM drain runs with hidden chain waits."""
                nonlocal v2
                c0, n = blocks[bi]
                m2 = m2_tiles[bi]
                HB = BL // 2
                vp = v2[:, 0:HB]
                vq = v2[:, HB:BL]
                for tl in range(n // BL):
                    mp = m2[:, tl * BL:tl * BL + HB]
                    mq = m2[:, tl * BL + HB:(tl + 1) * BL]
                    nc.vector.scalar_tensor_tensor(
                        out=mp, in0=vp, scalar=-BETA,
                        in1=c2[:, c0 + tl * BL:c0 + tl * BL + HB],
                        op0=Alu.mult, op1=Alu.add,
                    )
                    nc.vector.scalar_tensor_tensor(
                        out=mq, in0=vq, scalar=-BETA,
                        in1=c2[:, c0 + tl * BL + HB:c0 + (tl + 1) * BL],
                        op0=Alu.mult, op1=Alu.add,
                    )
                    vpn = mvpool.tile([O, HB], f32, tag="vp")
                    nc.vector.scalar_tensor_tensor(
                        out=vpn[:], in0=mp, scalar=BETA, in1=mp,
                        op0=Alu.is_gt, op1=Alu.subtract,
                    )
                    vqn = mvpool.tile([O, HB], f32, tag="vq")
                    nc.vector.scalar_tensor_tensor(
                        out=vqn[:], in0=mq, scalar=BETA, in1=mq,
                        op0=Alu.is_gt, op1=Alu.subtract,
                    )
                    vp = vpn[:]
                    vq = vqn[:]

            def out2(bi):
                """spk2 (Vector) + 1/beta un-scale (Scalar) for block bi."""
                c0, n = blocks[bi]
                m2 = m2_tiles.pop(bi)
                s2b = opool.tile([O, CHUNK], f32, tag="s2b")
                nc.vector.tensor_scalar(
                    s2b[:, :n], m2[:, :n], BETA, None, Alu.is_gt,
                )
                nc.sync.dma_start(out=S2[:, c0:c0 + n], in_=s2b[:, :n])
                m2s = opool.tile([O, CHUNK], f32, tag="m2s")
                nc.scalar.activation(
                    out=m2s[:, :n], in_=m2[:, :n],
                    func=Act.Identity, bias=0.0, scale=1.0 / BETA,
                )
                nc.sync.dma_start(out=M2[:, c0:c0 + n], in_=m2s[:, :n])

            nb = len(blocks)
            for bi in range(nb):
                gemm1(bi)
                if bi > 0:
                    gemm2(bi - 1)
                scans(bi)            # scan1(bi) + scan2(bi-1) interleaved
                if bi > 1:
                    out2(bi - 2)
            gemm2(nb - 1)
            scan2_tail_split(nb - 1)
            out2(nb - 2)
            out2(nb - 1)

    nc.compile()
    return nc


def _get_nc():
    global _nc_cache
    if _nc_cache is None:
        _nc_cache = _build()
    return _nc_cache


def _split16(a):
    hi = np.asarray(a, np.float16)
    lo = np.asarray(a - hi.astype(np.float32), np.float16)
    return hi, lo


def _prep_shared(w1, b1, w2, b2):
    w1s = (BETA * w1).T.astype(np.float32)        # (784, 1024)
    w1h_f, w1l_f = _split16(w1s)

    # m-major weight layout: row m*128+p, col k*128+c = w1s[k*128+p, m*128+c]
    def mmajor(wf):
        return np.ascontiguousarray(
            wf[:768].reshape(KF, 128, HC, 128)      # (k, p, m, c)
            .transpose(2, 1, 0, 3)                  # (m, p, k, c)
            .reshape(HC * 128, KF * 128)
        )
    w1h = mmajor(w1h_f)
    w1l = mmajor(w1l_f)
    # packed 48-row tail: rows pair as (w1h,xh), (w1l,xh), (w1h,xl)
    w1t = np.ascontiguousarray(
        np.concatenate([w1h_f[768:], w1l_f[768:], w1h_f[768:]], axis=0)
    )
    b1v = np.ascontiguousarray((BETA * b1).astype(np.float32).reshape(HC, 128).T)
    # GEMM2 consumes sign-spikes s = 2*spk-1:  spk@w2.T = s@(w2/2).T + rowsum(w2)/2
    w2s = (0.5 * BETA * w2).T.astype(np.float32).reshape(HC, 128, O).transpose(1, 0, 2)
    w2p = np.ascontiguousarray(w2s.reshape(128, HC * O))
    b2v = (BETA * (b2 + 0.5 * w2.sum(axis=1))).astype(np.float32).reshape(O, 1)
    return w1h, w1l, w1t, b1v, w2p, b2v


def _make_in_maps(x, w1, b1, w2, b2):
    w1h, w1l, w1t, b1v, w2p, b2v = _prep_shared(w1, b1, w2, b2)
    in_maps = []
    for c in range(NCORES):
        xs = x[c * BL:(c + 1) * BL]                     # (BL, T, I)
        xT = np.ascontiguousarray(
            xs.transpose(2, 1, 0).reshape(I, TB)        # col = t*BL + b
        )
        xh_f, xl_f = _split16(xT)
        xh = np.ascontiguousarray(xh_f[:768])
        xl = np.ascontiguousarray(xl_f[:768])
        xt = np.ascontiguousarray(
            np.concatenate([xh_f[768:], xh_f[768:], xl_f[768:]], axis=0)
        )
        in_maps.append({
            "xh": xh, "xl": xl, "xt": xt, "w1h": w1h, "w1l": w1l, "w1t": w1t,
            "b1c": b1v, "w2p": w2p, "b2c": b2v,
        })
    return in_maps


def kernel(x, w1, b1, w2, b2):
    from concourse.bass_utils import run_bass_kernel_spmd

    nc = _get_nc()
    in_maps = _make_in_maps(x, w1, b1, w2, b2)
    res = run_bass_kernel_spmd(nc, in_maps, core_ids=list(range(NCORES)))

    spk = np.empty((B, T, O), np.float32)
    mem = np.empty((B, T, O), np.float32)
    for c in range(NCORES):
        r = res.results[c]
        spk[c * BL:(c + 1) * BL] = r["S2"].reshape(O, T, BL).transpose(2, 1, 0)
        mem[c * BL:(c + 1) * BL] = r["M2"].reshape(O, T, BL).transpose(2, 1, 0)
    return spk, mem
